# revision 8
# baseline (speedup 1.0000x reference)
"""Trainium2 Bass kernel for int8 GEMM + fp32 bias (linear_a8_w8_bfp32_ofp32).

Computes out = (x_int8 @ weight_int8.T).astype(f32) + bias  for
x [8192, 4096] int8, weight [4096, 4096] int8, bias [4096] f32.

Strategy: column-parallel tensor parallelism over 8 NeuronCores — each core
gets all of x (replicated) and a 512-column slice of weight/bias, and
computes its [8192, 512] output slice.

The PE array has no int8 matmul mode (TRN2/cayman dropped UINT8), but
int8 values are exactly representable in bf16, bf16 x bf16 products
(<= 127*127) are exact, and PSUM accumulates in fp32 where every partial
sum of this data stays far below 2^24 — so a bf16 matmul reproduces the
int32-accumulated reference bit-exactly. Inputs ship as int8 laid out
tile-contiguous by the host; x is cast to bf16 inside the DMA (SWDGE
casting DMA), w rides as raw int8 and is cast by the otherwise-idle DVE
at startup — both halve HBM/fabric traffic.

Startup is paced so the PE starts real matmuls as soon as the first w
chunk is cast (~1.3us after the DMA ring opens): the gpsimd ring is
ordered [w0, x0g0, w1, w2, x0g1, ...] so the k-tiles of m-tile 0 arrive
in consumption order, and m-tile 0's matmuls self-pace against the cast/
DMA semaphores while the HAM clock ramps.  No warmup matmuls — the real
stream does the warming.  The last m-tile is computed as two 256-column
PSUM chains so the final bias-add + store tail is halved.

Steady state: 64 m-tiles x 32 k-tiles of [128,128] x [128,512] matmuls
accumulating into one PSUM bank per m-tile; epilogue is a single DVE
tensor_add (PSUM + broadcast bias -> SBUF) and a contiguous store.
"""

import numpy as np

import concourse.mybir as mybir
import concourse.tile as tile
from concourse import bacc
from concourse.bass_utils import run_bass_kernel_spmd

P = 128
N_CORES = 8

# Set by a test harness to capture timing/trace info; harmless defaults.
TRACE = False
TRACE_KWARGS = {}
LAST_RESULT = None


def build_program(MT, KT, NLOC, x_bufs=4, o_bufs=3, psum_bufs=4, w_chunks=8):
    """Bass/Tile program for one core: out[MT*128, NLOC] = xT.T @ wT + bias.

    DRAM layouts (host pre-arranged, all contiguous per SBUF partition):
      x_tiles   [MT, P, KT, P]  int8   x_tiles[mt, ki, kt, mi] = x[mt*P+mi, kt*P+ki]
      w_tiles   [P, KT, NLOC]   int8   w_tiles[ki, kt, n] = weight[n, kt*P+ki]
      bias_bcast[P, NLOC]       f32    bias replicated across partitions
      out_tiles [MT, P, NLOC]   f32    out_tiles[mt, mi, n] = out[mt*P+mi, n]
    """
    assert KT % w_chunks == 0
    KC = KT // w_chunks  # k-tiles per w chunk
    XG = 4  # x(mt=0) arrives in XG chunks of KT//XG k-tiles
    KX = KT // XG
    nc = bacc.Bacc()
    BF_CHUNKS = 2  # leading w chunks shipped pre-cast as bf16 by the host
    x_d = nc.declare_dram_parameter(
        "x_tiles", [MT, P, KT, P], mybir.dt.int8, isOutput=False
    )
    w_d = nc.declare_dram_parameter(
        "w_tiles", [P, KT, NLOC], mybir.dt.int8, isOutput=False
    )
    wbf_d = nc.declare_dram_parameter(
        "w_head_bf16", [P, BF_CHUNKS * KC, NLOC], mybir.dt.bfloat16, isOutput=False
    )
    b_d = nc.declare_dram_parameter(
        "bias_bcast", [P, NLOC], mybir.dt.float32, isOutput=False
    )
    o_d = nc.declare_dram_parameter(
        "out_tiles", [MT, P, NLOC], mybir.dt.float32, isOutput=True
    )

    with tile.TileContext(nc) as tc:
        with (
            tc.tile_pool(name="wpool", bufs=1) as wpool,
            tc.tile_pool(name="wqpool", bufs=8) as wqpool,
            tc.tile_pool(name="cpool", bufs=1) as cpool,
            tc.tile_pool(name="xpool", bufs=x_bufs) as xpool,
            tc.tile_pool(name="opool", bufs=o_bufs) as opool,
            tc.tile_pool(name="psum", bufs=psum_bufs, space="PSUM") as psum_pool,
            tc.tile_pool(name="psum2", bufs=2, space="PSUM") as psum2_pool,
            tc.tile_pool(name="warm", bufs=1) as warm_pool,
            tc.tile_pool(name="warm_ps", bufs=1, space="PSUM") as warm_psum,
        ):
            # PE warmup: matmuls on a zeroed tile, dependent on nothing but a
            # gpsimd memset, keep the PE busy from ~7.3us so the HAM clock
            # un-throttles (~5us of sustained activity) before real matmuls
            # start at ~11us.
            warmup_mms = 7
            wu = warm_pool.tile([P, NLOC], mybir.dt.bfloat16)
            nc.vector.memset(wu[:], 0.0)
            wu_ps = warm_psum.tile([P, NLOC], mybir.dt.float32)
            for i in range(warmup_mms):
                nc.tensor.matmul(
                    wu_ps[:],
                    wu[:, :P],
                    wu[:],
                    start=(i == 0),
                    stop=(i == warmup_mms - 1),
                )

            # Startup loads: w chunks ride the otherwise-idle SYNC queue
            # (int8 staging + DVE cast) while x(mt=0)/x(mt=1) k-groups ride
            # the gpsimd SWDGE casting ring — descriptor generation for the
            # two streams proceeds in parallel, so w0 is cast and the first
            # real matmul issues ~4us after the DMA rings open.  The bias
            # load rides at the BACK of the sync queue (it isn't needed
            # until the first bias-add ~14us later) so it doesn't steal DMA
            # engine time from the critical startup path.
            w_sb = []
            for j in range(w_chunks):
                w_c = wpool.tile([P, KC, NLOC], mybir.dt.bfloat16, tag=f"w{j}")
                w_sb.append(w_c)
                if j < BF_CHUNKS:
                    # head chunks arrive pre-cast: straight to bf16 SBUF,
                    # no staging buffer and no DVE cast on the critical path
                    nc.sync.dma_start(
                        out=w_c[:], in_=wbf_d[:, j * KC : (j + 1) * KC, :]
                    )
                else:
                    wq = wqpool.tile([P, KC, NLOC], mybir.dt.int8)
                    nc.sync.dma_start(
                        out=wq[:], in_=w_d[:, j * KC : (j + 1) * KC, :]
                    )
                    nc.vector.tensor_copy(w_c[:], wq[:])

            x0_sb = []
            x1_sb = []
            for g in range(XG):
                x_c = xpool.tile([P, KX, P], mybir.dt.bfloat16, tag=f"x0g{g}", bufs=1)
                nc.gpsimd.dma_start(
                    out=x_c[:], in_=x_d[0, :, g * KX : (g + 1) * KX, :]
                )
                x0_sb.append(x_c)
                if MT > 1:
                    x_c = xpool.tile(
                        [P, KX, P], mybir.dt.bfloat16, tag=f"x1g{g}", bufs=1
                    )
                    nc.gpsimd.dma_start(
                        out=x_c[:], in_=x_d[1, :, g * KX : (g + 1) * KX, :]
                    )
                    x1_sb.append(x_c)

            b_sb = cpool.tile([P, NLOC], mybir.dt.float32)
            nc.sync.dma_start(out=b_sb[:], in_=b_d[:])

            def lhsT_for(mt, kt):
                if mt == 0:
                    return x0_sb[kt // KX][:, kt % KX, :]
                if mt == 1 and x1_sb:
                    return x1_sb[kt // KX][:, kt % KX, :]
                return x_cur[:, kt, :]

            # m-tiles 0 and 1 run as one k-interleaved ping-pong pair: each
            # k-tile of w feeds two matmuls back-to-back, halving the pace at
            # which the startup DMA/cast pipeline must deliver w — the PE
            # stays busy while w streams in, instead of stalling at the tail
            # of m-tile 0.
            if MT > 1:
                ps0 = psum_pool.tile([P, NLOC], mybir.dt.float32, tag="ps")
                ps1 = psum_pool.tile([P, NLOC], mybir.dt.float32, tag="ps")
                for kt in range(KT):
                    for mt, ps in ((0, ps0), (1, ps1)):
                        nc.tensor.matmul(
                            ps[:],
                            lhsT_for(mt, kt),
                            w_sb[kt // KC][:, kt % KC, :],
                            start=(kt == 0),
                            stop=(kt == KT - 1),
                        )
                for mt, ps in ((0, ps0), (1, ps1)):
                    o_sb = opool.tile([P, NLOC], mybir.dt.float32)
                    nc.vector.tensor_add(o_sb[:], ps[:], b_sb[:])
                    nc.sync.dma_start(out=o_d[mt], in_=o_sb[:])

            for mt in range(2 if MT > 1 else 0, MT):
                x_cur = xpool.tile([P, KT, P], mybir.dt.bfloat16)
                nc.gpsimd.dma_start(out=x_cur[:], in_=x_d[mt])
                if mt == MT - 1:
                    # Last m-tile: two 256-wide chains so the final
                    # bias-add + store tail is half-width.
                    NH = NLOC // 2
                    for h in range(2):
                        ps = psum2_pool.tile([P, NH], mybir.dt.float32)
                        sl = slice(h * NH, (h + 1) * NH)
                        for kt in range(KT):
                            nc.tensor.matmul(
                                ps[:],
                                lhsT_for(mt, kt),
                                w_sb[kt // KC][:, kt % KC, sl],
                                start=(kt == 0),
                                stop=(kt == KT - 1),
                            )
                        o_sb = opool.tile([P, NH], mybir.dt.float32, tag=f"oh{h}")
                        nc.vector.tensor_add(o_sb[:], ps[:], b_sb[:, sl])
                        nc.sync.dma_start(out=o_d[mt, :, sl], in_=o_sb[:])
                else:
                    ps = psum_pool.tile([P, NLOC], mybir.dt.float32)
                    for kt in range(KT):
                        nc.tensor.matmul(
                            ps[:],
                            lhsT_for(mt, kt),
                            w_sb[kt // KC][:, kt % KC, :],
                            start=(kt == 0),
                            stop=(kt == KT - 1),
                        )
                    o_sb = opool.tile([P, NLOC], mybir.dt.float32)
                    nc.vector.tensor_add(o_sb[:], ps[:], b_sb[:])
                    nc.sync.dma_start(out=o_d[mt], in_=o_sb[:])
    nc.compile()
    return nc


def run(x, weight, fake_bias):
    global LAST_RESULT
    M, K = x.shape
    N = weight.shape[0]
    assert M % P == 0 and K % P == 0 and N % (N_CORES * P) == 0
    MT, KT, NLOC = M // P, K // P, N // N_CORES

    import ml_dtypes

    xb = np.asarray(x).astype(np.int8)
    x_tiles = np.ascontiguousarray(xb.reshape(MT, P, KT, P).transpose(0, 3, 2, 1))
    wb = np.asarray(weight).astype(np.int8)
    bias = np.asarray(fake_bias).astype(np.float32)

    KC = KT // 8
    in_maps = []
    for c in range(N_CORES):
        w_loc = wb[c * NLOC : (c + 1) * NLOC, :]  # [NLOC, K]
        w_tiles = np.ascontiguousarray(
            w_loc.T.reshape(KT, P, NLOC).transpose(1, 0, 2)
        )
        w_head = np.ascontiguousarray(
            w_tiles[:, : 2 * KC, :].astype(ml_dtypes.bfloat16)
        )
        b_loc = np.ascontiguousarray(
            np.broadcast_to(bias[None, c * NLOC : (c + 1) * NLOC], (P, NLOC))
        )
        in_maps.append(
            {
                "x_tiles": x_tiles,
                "w_tiles": w_tiles,
                "w_head_bf16": w_head,
                "bias_bcast": b_loc,
            }
        )

    nc = build_program(MT, KT, NLOC)
    res = run_bass_kernel_spmd(
        nc, in_maps, list(range(N_CORES)), trace=TRACE, **TRACE_KWARGS
    )
    LAST_RESULT = res

    outs = [r["out_tiles"].reshape(M, NLOC) for r in res.results]
    return np.concatenate(outs, axis=1).astype(np.float32)


def kernel(x, weight, fake_bias):
    return run(x, weight, fake_bias)


# revision 14
# speedup vs baseline: 1.0045x; 1.0045x over previous
"""Trainium2 Bass kernel for int8 GEMM + fp32 bias (linear_a8_w8_bfp32_ofp32).

Computes out = (x_int8 @ weight_int8.T).astype(f32) + bias  for
x [8192, 4096] int8, weight [4096, 4096] int8, bias [4096] f32.

Strategy: column-parallel tensor parallelism over 8 NeuronCores — each core
gets all of x (replicated) and a 512-column slice of weight/bias, and
computes its [8192, 512] output slice.

The PE array has no int8 matmul mode (TRN2/cayman dropped UINT8), but
int8 values are exactly representable in bf16, bf16 x bf16 products
(<= 127*127) are exact, and PSUM accumulates in fp32 where every partial
sum of this data stays far below 2^24 — so a bf16 matmul reproduces the
int32-accumulated reference bit-exactly. Inputs ship as int8 laid out
tile-contiguous by the host; x is cast to bf16 inside the DMA (SWDGE
casting DMA), w rides as raw int8 and is cast by the otherwise-idle DVE
at startup — both halve HBM/fabric traffic.

Startup is paced so the PE starts real matmuls as soon as the first w
chunk is cast (~1.3us after the DMA ring opens): the gpsimd ring is
ordered [w0, x0g0, w1, w2, x0g1, ...] so the k-tiles of m-tile 0 arrive
in consumption order, and m-tile 0's matmuls self-pace against the cast/
DMA semaphores while the HAM clock ramps.  No warmup matmuls — the real
stream does the warming.  The last m-tile is computed as two 256-column
PSUM chains so the final bias-add + store tail is halved.

Steady state: 64 m-tiles x 32 k-tiles of [128,128] x [128,512] matmuls
accumulating into one PSUM bank per m-tile; epilogue is a single DVE
tensor_add (PSUM + broadcast bias -> SBUF) and a contiguous store.
"""

import numpy as np

import concourse.mybir as mybir
import concourse.tile as tile
from concourse import bacc
from concourse.bass_utils import run_bass_kernel_spmd

P = 128
N_CORES = 8

# Set by a test harness to capture timing/trace info; harmless defaults.
TRACE = False
TRACE_KWARGS = {}
LAST_RESULT = None


def build_program(MT, KT, NLOC, x_bufs=4, o_bufs=3, psum_bufs=4, w_chunks=8):
    """Bass/Tile program for one core: out[MT*128, NLOC] = xT.T @ wT + bias.

    DRAM layouts (host pre-arranged, all contiguous per SBUF partition):
      x_tiles   [MT, P, KT, P]  int8   x_tiles[mt, ki, kt, mi] = x[mt*P+mi, kt*P+ki]
      w_tiles   [P, KT, NLOC]   int8   w_tiles[ki, kt, n] = weight[n, kt*P+ki]
      bias_bcast[P, NLOC]       f32    bias replicated across partitions
      out_tiles [MT, P, NLOC]   f32    out_tiles[mt, mi, n] = out[mt*P+mi, n]
    """
    assert KT % w_chunks == 0
    KC = KT // w_chunks  # k-tiles per w chunk
    XG = 4  # x(mt=0) arrives in XG chunks of KT//XG k-tiles
    KX = KT // XG
    nc = bacc.Bacc()
    x_d = nc.declare_dram_parameter(
        "x_tiles", [MT, P, KT, P], mybir.dt.int8, isOutput=False
    )
    w_d = nc.declare_dram_parameter(
        "w_tiles", [P, KT, NLOC], mybir.dt.int8, isOutput=False
    )
    b_d = nc.declare_dram_parameter(
        "bias_bcast", [P, NLOC], mybir.dt.float32, isOutput=False
    )
    o_d = nc.declare_dram_parameter(
        "out_tiles", [MT, P, NLOC], mybir.dt.float32, isOutput=True
    )

    with tile.TileContext(nc) as tc:
        with (
            tc.tile_pool(name="wpool", bufs=1) as wpool,
            tc.tile_pool(name="wqpool", bufs=8) as wqpool,
            tc.tile_pool(name="cpool", bufs=1) as cpool,
            tc.tile_pool(name="xpool", bufs=x_bufs) as xpool,
            tc.tile_pool(name="opool", bufs=o_bufs) as opool,
            tc.tile_pool(name="psum", bufs=psum_bufs, space="PSUM") as psum_pool,
            tc.tile_pool(name="psum2", bufs=2, space="PSUM") as psum2_pool,
            tc.tile_pool(name="warm", bufs=1) as warm_pool,
            tc.tile_pool(name="warm_ps", bufs=1, space="PSUM") as warm_psum,
        ):
            # PE warmup: matmuls on a zeroed tile, dependent on nothing but a
            # DVE memset, keep the PE busy from ~7.3us so the HAM clock
            # un-throttles (~5us of sustained activity) by the time real
            # matmuls start at ~12.5us.
            wu = warm_pool.tile([P, NLOC], mybir.dt.bfloat16)
            nc.vector.memset(wu[:], 0.0)
            wu_ps = warm_psum.tile([P, NLOC], mybir.dt.float32)

            def warmup(n):
                for i in range(n):
                    nc.tensor.matmul(
                        wu_ps[:],
                        wu[:, :P],
                        wu[:],
                        start=(i == 0),
                        stop=(i == n - 1),
                    )

            warmup(11)

            # Startup loads: w chunks ride the otherwise-idle SYNC queue
            # (int8 staging + DVE cast) while x(mt=0)/x(mt=1) k-groups ride
            # the gpsimd SWDGE casting ring — descriptor generation for the
            # two streams proceeds in parallel.  x1's first k-group also
            # rides the sync queue (3rd slot) because the m0/m1 ping-pong
            # needs it early.  The bias load rides at the BACK of the sync
            # queue (it isn't needed until the first bias-add ~15us later)
            # so it doesn't steal DMA engine time from the critical path.
            w_sb = []
            x0_sb = []
            x1_sb = []

            def emit_w(j):
                w_c = wpool.tile([P, KC, NLOC], mybir.dt.bfloat16, tag=f"w{j}")
                w_sb.append(w_c)
                wq = wqpool.tile([P, KC, NLOC], mybir.dt.int8)
                nc.sync.dma_start(out=wq[:], in_=w_d[:, j * KC : (j + 1) * KC, :])
                nc.vector.tensor_copy(w_c[:], wq[:])

            def emit_x(mt, g, lst, queue):
                x_c = xpool.tile(
                    [P, KX, P], mybir.dt.bfloat16, tag=f"x{mt}g{g}", bufs=1
                )
                queue.dma_start(out=x_c[:], in_=x_d[mt, :, g * KX : (g + 1) * KX, :])
                lst.append(x_c)

            emit_w(0)
            emit_w(1)
            emit_x(0, 0, x0_sb, nc.gpsimd)
            if MT > 1:
                emit_x(1, 0, x1_sb, nc.gpsimd)
            emit_x(0, 1, x0_sb, nc.gpsimd)
            emit_w(2)
            emit_w(3)
            if MT > 1:
                emit_x(1, 1, x1_sb, nc.gpsimd)
            emit_x(0, 2, x0_sb, nc.gpsimd)
            emit_w(4)
            emit_w(5)
            if MT > 1:
                emit_x(1, 2, x1_sb, nc.gpsimd)
            emit_x(0, 3, x0_sb, nc.gpsimd)
            emit_w(6)
            emit_w(7)
            if MT > 1:
                emit_x(1, 3, x1_sb, nc.gpsimd)

            b_sb = cpool.tile([P, NLOC], mybir.dt.float32)
            nc.sync.dma_start(out=b_sb[:], in_=b_d[:])

            def lhsT_for(mt, kt):
                if mt == 0:
                    return x0_sb[kt // KX][:, kt % KX, :]
                if mt == 1 and x1_sb:
                    return x1_sb[kt // KX][:, kt % KX, :]
                return x_cur[:, kt, :]

            # m-tiles 0 and 1 run as a skewed k-interleaved ping-pong: m0
            # runs its first 4 k-tiles solo (only chunk 0 is needed), a
            # short warmup bridge keeps the PE active while x1g0/w1-cast
            # land, then each w k-tile feeds an (m0, m1) matmul pair --
            # halving the pace at which the startup DMA/cast pipeline must
            # deliver w so the PE never stalls on the w tail.
            if MT > 1:
                ps0 = psum_pool.tile([P, NLOC], mybir.dt.float32, tag="ps")
                ps1 = psum_pool.tile([P, NLOC], mybir.dt.float32, tag="ps")
                SKEW = 4

                def mm(mt, ps, kt):
                    nc.tensor.matmul(
                        ps[:],
                        lhsT_for(mt, kt),
                        w_sb[kt // KC][:, kt % KC, :],
                        start=(kt == 0),
                        stop=(kt == KT - 1),
                    )

                for kt in range(SKEW):
                    mm(0, ps0, kt)
                warmup(3)
                for j in range(KT - SKEW):
                    mm(0, ps0, SKEW + j)
                    mm(1, ps1, j)
                for kt in range(KT - SKEW, KT):
                    mm(1, ps1, kt)
                for mt, ps in ((0, ps0), (1, ps1)):
                    o_sb = opool.tile([P, NLOC], mybir.dt.float32)
                    nc.vector.tensor_add(o_sb[:], ps[:], b_sb[:])
                    nc.sync.dma_start(out=o_d[mt], in_=o_sb[:])

            for mt in range(2 if MT > 1 else 0, MT):
                x_cur = xpool.tile([P, KT, P], mybir.dt.bfloat16)
                nc.gpsimd.dma_start(out=x_cur[:], in_=x_d[mt])
                if mt == MT - 1:
                    # Last m-tile: a wide chain then a narrow 128-col chain,
                    # so the final bias-add + store tail is quarter-width
                    # while the extra per-matmul overhead stays small.
                    splits = [(0, 384), (384, 128)]
                    for h, (off, nw) in enumerate(splits):
                        ps = psum2_pool.tile(
                            [P, nw], mybir.dt.float32, tag=f"psl{h}", bufs=1
                        )
                        sl = slice(off, off + nw)
                        for kt in range(KT):
                            nc.tensor.matmul(
                                ps[:],
                                lhsT_for(mt, kt),
                                w_sb[kt // KC][:, kt % KC, sl],
                                start=(kt == 0),
                                stop=(kt == KT - 1),
                            )
                        o_sb = opool.tile([P, nw], mybir.dt.float32, tag=f"oh{h}")
                        nc.vector.tensor_add(o_sb[:], ps[:], b_sb[:, sl])
                        nc.sync.dma_start(out=o_d[mt, :, sl], in_=o_sb[:])
                else:
                    ps = psum_pool.tile([P, NLOC], mybir.dt.float32)
                    for kt in range(KT):
                        nc.tensor.matmul(
                            ps[:],
                            lhsT_for(mt, kt),
                            w_sb[kt // KC][:, kt % KC, :],
                            start=(kt == 0),
                            stop=(kt == KT - 1),
                        )
                    o_sb = opool.tile([P, NLOC], mybir.dt.float32)
                    nc.vector.tensor_add(o_sb[:], ps[:], b_sb[:])
                    nc.sync.dma_start(out=o_d[mt], in_=o_sb[:])
    nc.compile()
    return nc


def run(x, weight, fake_bias):
    global LAST_RESULT
    M, K = x.shape
    N = weight.shape[0]
    assert M % P == 0 and K % P == 0 and N % (N_CORES * P) == 0
    MT, KT, NLOC = M // P, K // P, N // N_CORES

    xb = np.asarray(x).astype(np.int8)
    x_tiles = np.ascontiguousarray(xb.reshape(MT, P, KT, P).transpose(0, 3, 2, 1))
    wb = np.asarray(weight).astype(np.int8)
    bias = np.asarray(fake_bias).astype(np.float32)

    in_maps = []
    for c in range(N_CORES):
        w_loc = wb[c * NLOC : (c + 1) * NLOC, :]  # [NLOC, K]
        w_tiles = np.ascontiguousarray(
            w_loc.T.reshape(KT, P, NLOC).transpose(1, 0, 2)
        )
        b_loc = np.ascontiguousarray(
            np.broadcast_to(bias[None, c * NLOC : (c + 1) * NLOC], (P, NLOC))
        )
        in_maps.append(
            {"x_tiles": x_tiles, "w_tiles": w_tiles, "bias_bcast": b_loc}
        )

    nc = build_program(MT, KT, NLOC)
    res = run_bass_kernel_spmd(
        nc, in_maps, list(range(N_CORES)), trace=TRACE, **TRACE_KWARGS
    )
    LAST_RESULT = res

    outs = [r["out_tiles"].reshape(M, NLOC) for r in res.results]
    return np.concatenate(outs, axis=1).astype(np.float32)


def kernel(x, weight, fake_bias):
    return run(x, weight, fake_bias)
